# revision 13
# baseline (speedup 1.0000x reference)
"""GQA attention (llama3-style RoPE, causal) on 8 trn2 NeuronCores.

Sharding: tensor-parallel over KV-head groups for QKV+attention; the o_proj
is COLUMN-parallel. Core i owns kv-head i and its 4 query heads:
wq[:, i*512:(i+1)*512], wk/wv[:, i*128:(i+1)*128], plus the COLUMN slice
wo[:, i*512:(i+1)*512]. After attention, chunked AllGathers (bf16, one per
tq-block of 512 and head-PAIR) distribute every core's head-transposed
attention output [256, 512] -> gathered [2048, 512]; each core then computes
its 512 output columns for ALL rows (out[t, i*512:(i+1)*512]) with no
further collective. The host concatenates column blocks.

Dataflow per core (everything transposed-by-design, no PE transposes):
  proj:  qT/kT [d=128, T] = w-tile.T @ xT-tile, bf16 weights/x, fp32 PSUM,
         3 PSUM banks (two half-sweeps per tq-block; x streamed twice;
         v-half FIRST so its DRAM transpose round-trip hides)
  RoPE:  ACT does the half-swap, DVE the cos/sin muls; outputs bf16
  vT:    staged to DRAM bf16, read back via DMA-transpose -> v [t, d]
  sT  [tk, tq] = k-tile @ qT      (bf16)
  pT  = exp(sT/sqrt(d)) (ACT, bf16 out) * causal-mask (DVE)
  l   = ones.T @ pT               (M=1 matmul, fp32 PSUM accum over tk)
  oT  [d, tq] = v.T @ pT          (fp32 PSUM accum over tk)
  oT_norm = oT * (1/l)            (approx-recip + partition-broadcast), bf16
  oproj: out[t, 0:512] += agT-tile.T @ wo-tile  (bf16, fp32 PSUM, 32 k-tiles)
All matmuls bf16. Bulk DMAs are batched into >=0.5 MiB strided transfers
(weights/x/gathered/out) to amortize the ~0.6us per-DMA issue cost and hit
line-rate; issue is split across both HWDGE queues plus GpSimd SWDGE.
"""

import numpy as np

H, KV, HD, HID = 32, 8, 128, 4096
T = 2048
N_CORES = 8
QH = H // KV            # 4 query heads per core
DQ = QH * HD            # 512
KT = HID // 128         # 32 contraction tiles for projections
TN = T // 128           # 16 sequence tiles
G = 4                   # tq groups of 512
GW = T // G             # 512
NO = HID // N_CORES     # 512 output columns per core
XC = 8                  # k-tiles per x chunk DMA
WQC = 4                 # k-tiles per wq chunk DMA

THETA, FACTOR, HI_FF, LO_FF, ORIG_MAX = 500000.0, 8.0, 4.0, 1.0, 8192

_CACHE = {}


def _rope_tables():
    inv = 1.0 / (THETA ** (np.arange(0, HD, 2, dtype=np.float64) / HD))
    wavelen = 2.0 * np.pi / inv
    low_wl = ORIG_MAX / LO_FF
    high_wl = ORIG_MAX / HI_FF
    smooth = (ORIG_MAX / wavelen - LO_FF) / (HI_FF - LO_FF)
    scaled = np.where(wavelen > low_wl, inv / FACTOR, inv)
    mid = (wavelen <= low_wl) & (wavelen >= high_wl)
    scaled = np.where(mid, (1 - smooth) * inv / FACTOR + smooth * inv, scaled)
    inv32 = scaled.astype(np.float32)
    pos = np.arange(T, dtype=np.float32)
    freqs = pos[:, None] * inv32[None, :]          # [T, 64]
    emb = np.concatenate([freqs, freqs], axis=-1)  # [T, 128]
    cosT = np.ascontiguousarray(np.cos(emb).T)     # [128, T]
    sinT = np.ascontiguousarray(np.sin(emb).T)
    return cosT, sinT


def _causal_masks():
    # pT tile is [tk(part) 128, tq(free) 512]; within a tq-group the diagonal
    # tile sits at block v (=tk_tile - 4*g). keep where tq >= tk.
    tri = np.triu(np.ones((128, 128), dtype=np.float32))
    masks = np.zeros((4, 128, 512), dtype=np.float32)
    for v in range(4):
        for c in range(4):
            if c > v:
                masks[v, :, c * 128:(c + 1) * 128] = 1.0
            elif c == v:
                masks[v, :, c * 128:(c + 1) * 128] = tri
    return masks


def _build_program():
    import concourse.bacc as bacc
    import concourse.mybir as mybir
    from concourse.tile import TileContext

    f32 = mybir.dt.float32
    bf16 = mybir.dt.bfloat16
    EXPF = mybir.ActivationFunctionType.Exp

    nc = bacc.Bacc("TRN2", target_bir_lowering=False, debug=False,
                   num_devices=N_CORES)

    # weight DRAM tensors are pre-tiled on the host into [128, ...] layouts
    # so each loads with ONE contiguous line-rate DMA.
    xT = nc.dram_tensor("xT", [HID, T], bf16, kind="ExternalInput")
    wqd = nc.dram_tensor("wq", [128, KT * DQ], bf16, kind="ExternalInput")
    wkd = nc.dram_tensor("wk", [128, KT * HD], bf16, kind="ExternalInput")
    wvd = nc.dram_tensor("wv", [128, KT * HD], bf16, kind="ExternalInput")
    wod = nc.dram_tensor("wo", [128, KT * NO], bf16, kind="ExternalInput")
    cosd = nc.dram_tensor("cosT", [HD, T], f32, kind="ExternalInput")
    sind = nc.dram_tensor("sinT", [HD, T], f32, kind="ExternalInput")
    maskd = nc.dram_tensor("masks", [4, HD, GW], bf16, kind="ExternalInput")
    onesd = nc.dram_tensor("ones", [128, 1], bf16, kind="ExternalInput")
    outd = nc.dram_tensor("out", [T, NO], f32, kind="ExternalOutput")

    vstage = nc.dram_tensor("vstage", [HD, T], bf16)
    # one AllGather per tq-block: in [512, 512] -> out [4096, 512]
    ag_ins = [nc.dram_tensor(f"ag_in{g}", [DQ, GW], bf16) for g in range(G)]
    ag_outs = [nc.dram_tensor(f"ag_out{g}", [DQ * N_CORES, GW], bf16,
                              addr_space="Shared") for g in range(G)]

    scale = float(1.0 / np.sqrt(HD))

    with TileContext(nc) as tc:
        with (
            tc.tile_pool(name="const", bufs=1) as cpool,
            tc.tile_pool(name="wres", bufs=1) as wres,
            tc.tile_pool(name="stream", bufs=2) as stp,
            tc.tile_pool(name="qkv", bufs=2) as qkv,
            tc.tile_pool(name="kvres", bufs=1) as kvres,
            tc.tile_pool(name="rope", bufs=2) as rtp,
            tc.tile_pool(name="pt", bufs=4) as ptp,
            tc.tile_pool(name="norm", bufs=2) as nrm,
            tc.tile_pool(name="agbuf", bufs=1) as agp,
            tc.tile_pool(name="obuf", bufs=1) as obp,
            tc.tile_pool(name="ppsum", bufs=3, space="PSUM") as pps,
            tc.tile_pool(name="spsum", bufs=2, space="PSUM") as sps_pool,
            tc.tile_pool(name="opsum", bufs=1, space="PSUM") as ops_pool,
            tc.tile_pool(name="lpsum", bufs=1, space="PSUM") as lps_pool,
            tc.tile_pool(name="ojpsum", bufs=1, space="PSUM") as ojp,
        ):
            ones = cpool.tile([128, 1], bf16, tag="ones")
            mtiles = [cpool.tile([HD, GW], bf16, tag=f"mask{v}", name=f"mask{v}")
                      for v in range(4)]
            # resident weights; wq in 4-ktile chunks (fast first arrival),
            # wk/wv/wo as single big tiles loaded with one DMA each.
            wq_c = [wres.tile([128, WQC * DQ], bf16, tag=f"wq{c}", name=f"wq{c}")
                    for c in range(KT // WQC)]
            wkb = wres.tile([128, KT * HD], bf16, tag="wkb")
            wvb = wres.tile([128, KT * HD], bf16, tag="wvb")
            wob = wres.tile([128, KT * NO], bf16, tag="wob")
            kT_t = [kvres.tile([128, GW], bf16, tag=f"kT{g}", name=f"kT{g}") for g in range(G)]
            v_t = [kvres.tile([128, 128], bf16, tag=f"v{j}", name=f"v{j}") for j in range(TN)]

            warm = rtp.tile([1, 2], f32, tag="warm")
            nc.vector.memset(warm[:], 0.0)
            nc.scalar.activation(warm[:], warm[:], EXPF)

            def wq_ap(k, h):
                c, kk = k // WQC, k % WQC
                return wq_c[c][:, kk * DQ + h * 128:kk * DQ + (h + 1) * 128]

            def rope_drain(dst, src, cg, sg):
                # dst[bf16] = src*cos + rotate_half(src)*sin ; src is fp32 PSUM
                rot = rtp.tile([128, GW], f32, tag="rot")
                nc.scalar.mul(rot[0:64, :], src[64:128, :], -1.0)
                nc.scalar.copy(rot[64:128, :], src[0:64, :])
                t1 = rtp.tile([128, GW], f32, tag="t1")
                nc.vector.tensor_mul(t1[:], src[:], cg[:])
                nc.vector.tensor_mul(rot[:], rot[:], sg[:])
                nc.vector.tensor_add(dst, t1[:], rot[:])

            def oproj(g):
                # out[t, :NO] for t-block g from gathered attnT (all heads).
                # gathered block b (rows 128b) holds global head b; pair it
                # with wo k-tile b. One 4MB DMA fetches the whole gather.
                at = agp.tile([128, 32 * GW], bf16, tag="ag", name=f"ag{g}")
                nc.sync.dma_start(
                    at[:].rearrange("p (b t) -> p b t", b=32),
                    ag_outs[g][:].rearrange("(b p) t -> p b t", p=128))
                ob = obp.tile([128, 4 * NO], f32, tag="ob")
                for t in range(4):
                    opj = ojp.tile([128, NO], f32, tag="oj", name=f"oj{g}_{t}")
                    for b in range(32):
                        lhs = at[:, b * GW + t * 128:b * GW + (t + 1) * 128]
                        nc.tensor.matmul(opj[:], lhs, wob[:, b * NO:(b + 1) * NO],
                                         start=(b == 0), stop=(b == 31))
                    nc.vector.tensor_copy(ob[:, t * NO:(t + 1) * NO], opj[:])
                nc.scalar.dma_start(
                    outd[g * GW:(g + 1) * GW, :].rearrange("(tt p) n -> p tt n",
                                                           p=128),
                    ob[:].rearrange("p (tt n) -> p tt n", tt=4))

            qT_cur = [None] * QH   # per-g rotating qT tiles
            for g in range(G):
                gs = slice(g * GW, (g + 1) * GW)
                # ---- projections: two half-sweeps over 3 PSUM banks.
                # v-half FIRST: its DRAM round-trip hides under the k-half.
                for half in range(2):
                    h0, h1 = (2, 3) if half == 0 else (0, 1)
                    pp = [pps.tile([128, GW], f32, tag="pp", name=f"pp{g}_{half}_{_i}")
                          for _i in range(3)]
                    for k in range(KT):
                        if k % XC == 0:
                            xt = stp.tile([128, XC * GW], bf16, tag="xt")
                            nc.sync.dma_start(
                                xt[:].rearrange("p (c t) -> p c t", c=XC),
                                xT[k * 128:(k + XC) * 128, gs].rearrange(
                                    "(c p) t -> p c t", p=128))
                        if half == 0 and k == 16:
                            cg = rtp.tile([HD, GW], f32, tag="cosg")
                            sg = rtp.tile([HD, GW], f32, tag="sing")
                            nc.sync.dma_start(cg[:], cosd[:, gs])
                            nc.sync.dma_start(sg[:], sind[:, gs])
                        if g == 0 and half == 0 and k % WQC == 0:
                            nc.scalar.dma_start(
                                wq_c[k // WQC][:],
                                wqd[:, k * DQ:(k + WQC) * DQ])
                            if k == 0:
                                nc.scalar.dma_start(wvb[:], wvd[:])
                            if k == WQC:
                                nc.scalar.dma_start(wkb[:], wkd[:])
                        xs = xt[:, (k % XC) * GW:(k % XC + 1) * GW]
                        st, sp = (k == 0), (k == KT - 1)
                        wkv = (wvb if half == 0 else wkb)[:, k * HD:(k + 1) * HD]
                        nc.tensor.matmul(pp[0][:], wkv, xs, start=st, stop=sp)
                        nc.tensor.matmul(pp[1][:], wq_ap(k, h0), xs, start=st, stop=sp)
                        nc.tensor.matmul(pp[2][:], wq_ap(k, h1), xs, start=st, stop=sp)
                    # drains (v half first: DRAM round-trip ahead of it)
                    if half == 0:
                        vt = qkv.tile([128, GW], bf16, tag="vT")
                        nc.scalar.copy(vt[:], pp[0][:])
                        nc.gpsimd.dma_start(vstage[:, gs], vt[:])
                    for i, h in enumerate((h0, h1)):
                        qt = qkv.tile([128, GW], bf16, tag=f"qT{h}", name=f"qT{h}_{g}")
                        rope_drain(qt[:], pp[1 + i], cg, sg)
                        qT_cur[h] = qt
                    if half == 1:
                        rope_drain(kT_t[g][:], pp[0], cg, sg)
                        for ts in range(4):
                            j = 4 * g + ts
                            nc.sync.dma_start_transpose(
                                v_t[j][:], vstage[:, j * 128:(j + 1) * 128])
                    if g == 0 and half == 1:
                        nc.sync.dma_start(ones[:], onesd[:])
                        for v in range(4):
                            nc.sync.dma_start(mtiles[v][:], maskd[v])

                if g >= 2:
                    oproj(g - 2)

                # ---- attention for tq-block g; AG fires per head-pair ----
                nj = 4 * g + 4
                for h in range(QH):
                    ops_ = ops_pool.tile([128, GW], f32, tag="op")
                    lps = lps_pool.tile([1, GW], f32, tag="lp")

                    def lo_flush(jj, c0, pt):
                        nc.tensor.matmul(lps[:, c0:], ones[:], pt[:, c0:],
                                         start=(jj == 0), stop=(jj == nj - 1))
                        nc.tensor.matmul(ops_[:, c0:], v_t[jj][:], pt[:, c0:],
                                         start=(jj == 0), stop=(jj == nj - 1))

                    prev = None
                    for j in range(nj):
                        c0 = (j - 4 * g) * 128 if j >= 4 * g else 0
                        sps = sps_pool.tile([128, GW], f32, tag="sp")
                        nc.tensor.matmul(sps[:, c0:], kT_t[j // 4][:, (j % 4) * 128:(j % 4 + 1) * 128],
                                         qT_cur[h][:, c0:], start=True, stop=True)
                        pt = ptp.tile([128, GW], bf16, tag="pt")
                        nc.scalar.activation(pt[:, c0:], sps[:, c0:], EXPF, scale=scale)
                        if j >= 4 * g:
                            nc.vector.tensor_mul(pt[:, c0:], pt[:, c0:],
                                                 mtiles[j - 4 * g][:, c0:])
                        if prev is not None:
                            lo_flush(*prev)
                        prev = (j, c0, pt)
                    lo_flush(*prev)
                    ls = nrm.tile([1, GW], f32, tag="ls")
                    nc.vector.reciprocal_approx_fast(ls[:], lps[:])
                    lb = nrm.tile([128, GW], f32, tag="lb")
                    nc.gpsimd.partition_broadcast(lb[:], ls[:])
                    ot = qkv.tile([128, GW], bf16, tag=f"oT{h}")
                    nc.vector.tensor_mul(ot[:], ops_[:], lb[:])
                    nc.gpsimd.dma_start(ag_ins[g][h * 128:(h + 1) * 128, :], ot[:])
                    if h == QH - 1:
                        nc.gpsimd.collective_compute(
                            "AllGather", mybir.AluOpType.bypass,
                            replica_groups=[list(range(N_CORES))],
                            ins=[ag_ins[g][:]], outs=[ag_outs[g][:]],
                        )

                if g == 0:
                    nc.gpsimd.dma_start(wob[:], wod[:])
            oproj(G - 2)
            oproj(G - 1)

    nc.compile()
    return nc


def _get_program():
    if "nc" not in _CACHE:
        _CACHE["nc"] = _build_program()
    return _CACHE["nc"]


def _tile_rows(w, inner):
    # [HID, inner] -> [128, KT*inner]: k-tile k's rows land at cols k*inner
    return np.ascontiguousarray(
        w.reshape(KT, 128, inner).transpose(1, 0, 2).reshape(128, KT * inner))


def kernel(x, wq, wk, wv, wo):
    from concourse.bass_utils import run_bass_kernel_spmd
    from ml_dtypes import bfloat16

    nc = _get_program()

    x2 = np.asarray(x, dtype=np.float32).reshape(T, HID)
    xT = np.ascontiguousarray(x2.T).astype(bfloat16)
    cosT, sinT = _rope_tables()
    masks = _causal_masks().astype(bfloat16)
    ones = np.ones((128, 1), dtype=np.float32).astype(bfloat16)

    wq = np.asarray(wq, dtype=np.float32)
    wk = np.asarray(wk, dtype=np.float32)
    wv = np.asarray(wv, dtype=np.float32)
    wo = np.asarray(wo, dtype=np.float32)

    in_maps = []
    for i in range(N_CORES):
        in_maps.append({
            "xT": xT,
            "wq": _tile_rows(wq[:, i * DQ:(i + 1) * DQ], DQ).astype(bfloat16),
            "wk": _tile_rows(wk[:, i * HD:(i + 1) * HD], HD).astype(bfloat16),
            "wv": _tile_rows(wv[:, i * HD:(i + 1) * HD], HD).astype(bfloat16),
            "wo": _tile_rows(wo[:, i * NO:(i + 1) * NO], NO).astype(bfloat16),
            "cosT": cosT,
            "sinT": sinT,
            "masks": masks,
            "ones": ones,
        })

    _CACHE["last_in_maps"] = in_maps
    res = run_bass_kernel_spmd(nc, in_maps, list(range(N_CORES)))
    _CACHE["last_result"] = res
    out = np.empty((T, HID), dtype=np.float32)
    for i in range(N_CORES):
        out[:, i * NO:(i + 1) * NO] = res.results[i]["out"]
    return out.reshape(1, T, HID)


# revision 14
# speedup vs baseline: 1.0725x; 1.0725x over previous
"""GQA attention (llama3-style RoPE, causal) on 8 trn2 NeuronCores.

Sharding: tensor-parallel over KV-head groups for QKV+attention; the o_proj
is COLUMN-parallel. Core i owns kv-head i and its 4 query heads:
wq[:, i*512:(i+1)*512], wk/wv[:, i*128:(i+1)*128], plus the COLUMN slice
wo[:, i*512:(i+1)*512]. After attention, chunked AllGathers (bf16, one per
tq-block of 512 and head-PAIR) distribute every core's head-transposed
attention output [256, 512] -> gathered [2048, 512]; each core then computes
its 512 output columns for ALL rows (out[t, i*512:(i+1)*512]) with no
further collective. The host concatenates column blocks.

Dataflow per core (everything transposed-by-design, no PE transposes):
  proj:  qT/kT [d=128, T] = w-tile.T @ xT-tile, bf16 weights/x, fp32 PSUM,
         3 PSUM banks (two half-sweeps per tq-block; x streamed twice;
         v-half FIRST so its DRAM transpose round-trip hides)
  RoPE:  ACT does the half-swap, DVE the cos/sin muls; outputs bf16
  vT:    staged to DRAM bf16, read back via DMA-transpose -> v [t, d]
  sT  [tk, tq] = k-tile @ qT      (bf16)
  pT  = exp(sT/sqrt(d)) (ACT, bf16 out) * causal-mask (DVE)
  l   = ones.T @ pT               (M=1 matmul, fp32 PSUM accum over tk)
  oT  [d, tq] = v.T @ pT          (fp32 PSUM accum over tk)
  oT_norm = oT * (1/l)            (approx-recip + partition-broadcast), bf16
  oproj: out[t, 0:512] += agT-tile.T @ wo-tile  (bf16, fp32 PSUM, 32 k-tiles)
All matmuls bf16. Bulk DMAs are batched into >=0.5 MiB strided transfers
(weights/x/gathered/out) to amortize the ~0.6us per-DMA issue cost and hit
line-rate; issue is split across both HWDGE queues plus GpSimd SWDGE.
"""

import numpy as np

H, KV, HD, HID = 32, 8, 128, 4096
T = 2048
N_CORES = 8
QH = H // KV            # 4 query heads per core
DQ = QH * HD            # 512
KT = HID // 128         # 32 contraction tiles for projections
TN = T // 128           # 16 sequence tiles
G = 4                   # tq groups of 512
GW = T // G             # 512
NO = HID // N_CORES     # 512 output columns per core
XC = 8                  # k-tiles per x chunk DMA
WQC = 4                 # k-tiles per wq chunk DMA

THETA, FACTOR, HI_FF, LO_FF, ORIG_MAX = 500000.0, 8.0, 4.0, 1.0, 8192

_CACHE = {}


def _rope_tables():
    inv = 1.0 / (THETA ** (np.arange(0, HD, 2, dtype=np.float64) / HD))
    wavelen = 2.0 * np.pi / inv
    low_wl = ORIG_MAX / LO_FF
    high_wl = ORIG_MAX / HI_FF
    smooth = (ORIG_MAX / wavelen - LO_FF) / (HI_FF - LO_FF)
    scaled = np.where(wavelen > low_wl, inv / FACTOR, inv)
    mid = (wavelen <= low_wl) & (wavelen >= high_wl)
    scaled = np.where(mid, (1 - smooth) * inv / FACTOR + smooth * inv, scaled)
    inv32 = scaled.astype(np.float32)
    pos = np.arange(T, dtype=np.float32)
    freqs = pos[:, None] * inv32[None, :]          # [T, 64]
    emb = np.concatenate([freqs, freqs], axis=-1)  # [T, 128]
    cosT = np.ascontiguousarray(np.cos(emb).T)     # [128, T]
    sinT = np.ascontiguousarray(np.sin(emb).T)
    return cosT, sinT


def _causal_masks():
    # pT tile is [tk(part) 128, tq(free) 512]; within a tq-group the diagonal
    # tile sits at block v (=tk_tile - 4*g). keep where tq >= tk.
    tri = np.triu(np.ones((128, 128), dtype=np.float32))
    masks = np.zeros((4, 128, 512), dtype=np.float32)
    for v in range(4):
        for c in range(4):
            if c > v:
                masks[v, :, c * 128:(c + 1) * 128] = 1.0
            elif c == v:
                masks[v, :, c * 128:(c + 1) * 128] = tri
    return masks


def _build_program():
    import concourse.bacc as bacc
    import concourse.mybir as mybir
    from concourse.tile import TileContext

    f32 = mybir.dt.float32
    bf16 = mybir.dt.bfloat16
    EXPF = mybir.ActivationFunctionType.Exp

    nc = bacc.Bacc("TRN2", target_bir_lowering=False, debug=False,
                   num_devices=N_CORES)

    # weight DRAM tensors are pre-tiled on the host into [128, ...] layouts
    # so each loads with ONE contiguous line-rate DMA.
    xT = nc.dram_tensor("xT", [HID, T], bf16, kind="ExternalInput")
    wqd = nc.dram_tensor("wq", [128, KT * DQ], bf16, kind="ExternalInput")
    wkd = nc.dram_tensor("wk", [128, KT * HD], bf16, kind="ExternalInput")
    wvd = nc.dram_tensor("wv", [128, KT * HD], bf16, kind="ExternalInput")
    wod = nc.dram_tensor("wo", [128, KT * NO], bf16, kind="ExternalInput")
    cosd = nc.dram_tensor("cosT", [HD, T], f32, kind="ExternalInput")
    sind = nc.dram_tensor("sinT", [HD, T], f32, kind="ExternalInput")
    maskd = nc.dram_tensor("masks", [4, HD, GW], bf16, kind="ExternalInput")
    onesd = nc.dram_tensor("ones", [128, 1], bf16, kind="ExternalInput")
    outd = nc.dram_tensor("out", [T, NO], f32, kind="ExternalOutput")

    vstage = nc.dram_tensor("vstage", [HD, T], bf16)
    # one AllGather per (tq-block, head-pair): in [256,512] -> out [2048,512]
    ag_ins = [[nc.dram_tensor(f"ag_in{g}_{p}", [2 * HD, GW], bf16)
               for p in range(2)] for g in range(G)]
    ag_outs = [[nc.dram_tensor(f"ag_out{g}_{p}", [2 * HD * N_CORES, GW], bf16,
                               addr_space="Shared") for p in range(2)]
               for g in range(G)]
    wrm_in = nc.dram_tensor("wrm_in", [128, 16], bf16)
    wrm_out = nc.dram_tensor("wrm_out", [1024, 16], bf16, addr_space="Shared")

    scale = float(1.0 / np.sqrt(HD))

    with TileContext(nc) as tc:
        with (
            tc.tile_pool(name="const", bufs=1) as cpool,
            tc.tile_pool(name="wres", bufs=1) as wres,
            tc.tile_pool(name="stream", bufs=2) as stp,
            tc.tile_pool(name="qkv", bufs=2) as qkv,
            tc.tile_pool(name="kvres", bufs=1) as kvres,
            tc.tile_pool(name="rope", bufs=2) as rtp,
            tc.tile_pool(name="pt", bufs=4) as ptp,
            tc.tile_pool(name="norm", bufs=2) as nrm,
            tc.tile_pool(name="agbuf", bufs=1) as agp,
            tc.tile_pool(name="obuf", bufs=1) as obp,
            tc.tile_pool(name="ppsum", bufs=3, space="PSUM") as pps,
            tc.tile_pool(name="spsum", bufs=2, space="PSUM") as sps_pool,
            tc.tile_pool(name="opsum", bufs=1, space="PSUM") as ops_pool,
            tc.tile_pool(name="lpsum", bufs=1, space="PSUM") as lps_pool,
            tc.tile_pool(name="ojpsum", bufs=1, space="PSUM") as ojp,
        ):
            ones = cpool.tile([128, 1], bf16, tag="ones")
            mtiles = [cpool.tile([HD, GW], bf16, tag=f"mask{v}", name=f"mask{v}")
                      for v in range(4)]
            # resident weights; wq in 4-ktile chunks (fast first arrival),
            # wk/wv/wo as single big tiles loaded with one DMA each.
            wq_c = [wres.tile([128, WQC * DQ], bf16, tag=f"wq{c}", name=f"wq{c}")
                    for c in range(KT // WQC)]
            wkb = wres.tile([128, KT * HD], bf16, tag="wkb")
            wvb = wres.tile([128, KT * HD], bf16, tag="wvb")
            wob = wres.tile([128, KT * NO], bf16, tag="wob")
            kT_t = [kvres.tile([128, GW], bf16, tag=f"kT{g}", name=f"kT{g}") for g in range(G)]
            v_t = [kvres.tile([128, 128], bf16, tag=f"v{j}", name=f"v{j}") for j in range(TN)]

            warm = rtp.tile([1, 2], f32, tag="warm")
            nc.vector.memset(warm[:], 0.0)
            nc.scalar.activation(warm[:], warm[:], EXPF)
            wrs = rtp.tile([128, 16], bf16, tag="wrs")
            nc.vector.memset(wrs[:], 0.0)
            nc.gpsimd.dma_start(wrm_in[:], wrs[:])
            nc.gpsimd.collective_compute(
                "AllGather", mybir.AluOpType.bypass,
                replica_groups=[list(range(N_CORES))],
                ins=[wrm_in[:]], outs=[wrm_out[:]],
            )

            def wq_ap(k, h):
                c, kk = k // WQC, k % WQC
                return wq_c[c][:, kk * DQ + h * 128:kk * DQ + (h + 1) * 128]

            def rope_drain(dst, src, cg, sg):
                # dst[bf16] = src*cos + rotate_half(src)*sin ; src is fp32 PSUM
                rot = rtp.tile([128, GW], f32, tag="rot")
                nc.scalar.mul(rot[0:64, :], src[64:128, :], -1.0)
                nc.scalar.copy(rot[64:128, :], src[0:64, :])
                t1 = rtp.tile([128, GW], f32, tag="t1")
                nc.vector.tensor_mul(t1[:], src[:], cg[:])
                nc.vector.tensor_mul(rot[:], rot[:], sg[:])
                nc.vector.tensor_add(dst, t1[:], rot[:])

            def oproj(g):
                # out[t, :NO] for t-block g from gathered attnT (all heads).
                # pair-p block b=2c+hh holds global head 4c+2p+hh -> wo
                # k-tile 4c+2p+hh. One 2MB DMA per pair.
                ab = []
                for p in range(2):
                    at = agp.tile([128, 16 * GW], bf16, tag=f"ag{p}",
                                  name=f"ag{g}_{p}")
                    nc.sync.dma_start(
                        at[:].rearrange("p (b t) -> p b t", b=16),
                        ag_outs[g][p][:].rearrange("(b p) t -> p b t", p=128))
                    ab.append(at)
                order = [(p, c, hh) for p in range(2) for c in range(N_CORES)
                         for hh in range(2)]
                ob = obp.tile([128, 4 * NO], f32, tag="ob")
                for t in range(4):
                    opj = ojp.tile([128, NO], f32, tag="oj", name=f"oj{g}_{t}")
                    for fi, (p, c, hh) in enumerate(order):
                        lhs = ab[p][:, (2 * c + hh) * GW + t * 128:
                                    (2 * c + hh) * GW + (t + 1) * 128]
                        kw = 4 * c + 2 * p + hh
                        nc.tensor.matmul(opj[:], lhs, wob[:, kw * NO:(kw + 1) * NO],
                                         start=(fi == 0), stop=(fi == 31))
                    nc.vector.tensor_copy(ob[:, t * NO:(t + 1) * NO], opj[:])
                nc.scalar.dma_start(
                    outd[g * GW:(g + 1) * GW, :].rearrange("(tt p) n -> p tt n",
                                                           p=128),
                    ob[:].rearrange("p (tt n) -> p tt n", tt=4))

            qT_cur = [None] * QH   # per-g rotating qT tiles
            for g in range(G):
                gs = slice(g * GW, (g + 1) * GW)
                # ---- projections: two half-sweeps over 3 PSUM banks.
                # v-half FIRST: its DRAM round-trip hides under the k-half.
                for half in range(2):
                    h0, h1 = (2, 3) if half == 0 else (0, 1)
                    pp = [pps.tile([128, GW], f32, tag="pp", name=f"pp{g}_{half}_{_i}")
                          for _i in range(3)]
                    for k in range(KT):
                        if k % XC == 0:
                            xt = stp.tile([128, XC * GW], bf16, tag="xt")
                            nc.sync.dma_start(
                                xt[:].rearrange("p (c t) -> p c t", c=XC),
                                xT[k * 128:(k + XC) * 128, gs].rearrange(
                                    "(c p) t -> p c t", p=128))
                        if half == 0 and k == 16:
                            cg = rtp.tile([HD, GW], f32, tag="cosg")
                            sg = rtp.tile([HD, GW], f32, tag="sing")
                            nc.sync.dma_start(cg[:], cosd[:, gs])
                            nc.sync.dma_start(sg[:], sind[:, gs])
                        if g == 0 and half == 0 and k % WQC == 0:
                            if k == 0:
                                nc.scalar.dma_start(wvb[:], wvd[:])
                            nc.scalar.dma_start(
                                wq_c[k // WQC][:],
                                wqd[:, k * DQ:(k + WQC) * DQ])
                            if k == WQC:
                                nc.scalar.dma_start(wkb[:], wkd[:])
                        xs = xt[:, (k % XC) * GW:(k % XC + 1) * GW]
                        st, sp = (k == 0), (k == KT - 1)
                        nc.tensor.matmul(pp[1][:], wq_ap(k, h0), xs, start=st, stop=sp)
                        nc.tensor.matmul(pp[2][:], wq_ap(k, h1), xs, start=st, stop=sp)
                        wkv = (wvb if half == 0 else wkb)[:, k * HD:(k + 1) * HD]
                        nc.tensor.matmul(pp[0][:], wkv, xs, start=st, stop=sp)
                    # drains (v half first: DRAM round-trip ahead of it)
                    if half == 0:
                        vt = qkv.tile([128, GW], bf16, tag="vT")
                        nc.scalar.copy(vt[:], pp[0][:])
                        nc.gpsimd.dma_start(vstage[:, gs], vt[:])
                    for i, h in enumerate((h0, h1)):
                        qt = qkv.tile([128, GW], bf16, tag=f"qT{h}", name=f"qT{h}_{g}")
                        rope_drain(qt[:], pp[1 + i], cg, sg)
                        qT_cur[h] = qt
                    if half == 1:
                        rope_drain(kT_t[g][:], pp[0], cg, sg)
                        for ts in range(4):
                            j = 4 * g + ts
                            nc.sync.dma_start_transpose(
                                v_t[j][:], vstage[:, j * 128:(j + 1) * 128])
                    if g == 0 and half == 1:
                        nc.sync.dma_start(ones[:], onesd[:])
                        for v in range(4):
                            nc.sync.dma_start(mtiles[v][:], maskd[v])

                if g >= 2:
                    oproj(g - 2)

                # ---- attention for tq-block g; AG fires per head-pair ----
                nj = 4 * g + 4
                for h in range(QH):
                    ops_ = ops_pool.tile([128, GW], f32, tag="op")
                    lps = lps_pool.tile([1, GW], f32, tag="lp")

                    def lo_flush(jj, c0, pt):
                        nc.tensor.matmul(lps[:, c0:], ones[:], pt[:, c0:],
                                         start=(jj == 0), stop=(jj == nj - 1))
                        nc.tensor.matmul(ops_[:, c0:], v_t[jj][:], pt[:, c0:],
                                         start=(jj == 0), stop=(jj == nj - 1))

                    prev = None
                    for j in range(nj):
                        c0 = (j - 4 * g) * 128 if j >= 4 * g else 0
                        sps = sps_pool.tile([128, GW], f32, tag="sp")
                        nc.tensor.matmul(sps[:, c0:], kT_t[j // 4][:, (j % 4) * 128:(j % 4 + 1) * 128],
                                         qT_cur[h][:, c0:], start=True, stop=True)
                        pt = ptp.tile([128, GW], bf16, tag="pt")
                        nc.scalar.activation(pt[:, c0:], sps[:, c0:], EXPF, scale=scale)
                        if j >= 4 * g:
                            nc.vector.tensor_mul(pt[:, c0:], pt[:, c0:],
                                                 mtiles[j - 4 * g][:, c0:])
                        if prev is not None:
                            lo_flush(*prev)
                        prev = (j, c0, pt)
                    lo_flush(*prev)
                    ls = nrm.tile([1, GW], f32, tag="ls")
                    nc.vector.reciprocal_approx_fast(ls[:], lps[:])
                    lb = nrm.tile([128, GW], f32, tag="lb")
                    nc.gpsimd.partition_broadcast(lb[:], ls[:])
                    ot = qkv.tile([128, GW], bf16, tag=f"oT{h}")
                    nc.vector.tensor_mul(ot[:], ops_[:], lb[:])
                    nc.gpsimd.dma_start(ag_ins[g][h // 2][(h % 2) * 128:(h % 2 + 1) * 128, :],
                                        ot[:])
                    if h % 2 == 1:
                        p = h // 2
                        nc.gpsimd.collective_compute(
                            "AllGather", mybir.AluOpType.bypass,
                            replica_groups=[list(range(N_CORES))],
                            ins=[ag_ins[g][p][:]], outs=[ag_outs[g][p][:]],
                        )

                if g == 0:
                    nc.gpsimd.dma_start(wob[:], wod[:])
            oproj(G - 2)
            oproj(G - 1)

    nc.compile()
    return nc


def _get_program():
    if "nc" not in _CACHE:
        _CACHE["nc"] = _build_program()
    return _CACHE["nc"]


def _tile_rows(w, inner):
    # [HID, inner] -> [128, KT*inner]: k-tile k's rows land at cols k*inner
    return np.ascontiguousarray(
        w.reshape(KT, 128, inner).transpose(1, 0, 2).reshape(128, KT * inner))


def kernel(x, wq, wk, wv, wo):
    from concourse.bass_utils import run_bass_kernel_spmd
    from ml_dtypes import bfloat16

    nc = _get_program()

    x2 = np.asarray(x, dtype=np.float32).reshape(T, HID)
    xT = np.ascontiguousarray(x2.T).astype(bfloat16)
    cosT, sinT = _rope_tables()
    masks = _causal_masks().astype(bfloat16)
    ones = np.ones((128, 1), dtype=np.float32).astype(bfloat16)

    wq = np.asarray(wq, dtype=np.float32)
    wk = np.asarray(wk, dtype=np.float32)
    wv = np.asarray(wv, dtype=np.float32)
    wo = np.asarray(wo, dtype=np.float32)

    in_maps = []
    for i in range(N_CORES):
        in_maps.append({
            "xT": xT,
            "wq": _tile_rows(wq[:, i * DQ:(i + 1) * DQ], DQ).astype(bfloat16),
            "wk": _tile_rows(wk[:, i * HD:(i + 1) * HD], HD).astype(bfloat16),
            "wv": _tile_rows(wv[:, i * HD:(i + 1) * HD], HD).astype(bfloat16),
            "wo": _tile_rows(wo[:, i * NO:(i + 1) * NO], NO).astype(bfloat16),
            "cosT": cosT,
            "sinT": sinT,
            "masks": masks,
            "ones": ones,
        })

    _CACHE["last_in_maps"] = in_maps
    res = run_bass_kernel_spmd(nc, in_maps, list(range(N_CORES)))
    _CACHE["last_result"] = res
    out = np.empty((T, HID), dtype=np.float32)
    for i in range(N_CORES):
        out[:, i * NO:(i + 1) * NO] = res.results[i]["out"]
    return out.reshape(1, T, HID)


# revision 16
# speedup vs baseline: 1.0931x; 1.0193x over previous
"""GQA attention (llama3-style RoPE, causal) on 8 trn2 NeuronCores.

Sharding: tensor-parallel over KV-head groups for QKV+attention; the o_proj
is COLUMN-parallel. Core i owns kv-head i and its 4 query heads:
wq[:, i*512:(i+1)*512], wk/wv[:, i*128:(i+1)*128], plus the COLUMN slice
wo[:, i*512:(i+1)*512]. After attention, chunked AllGathers (bf16, one per
tq-block of 512 and head-PAIR) distribute every core's head-transposed
attention output [256, 512] -> gathered [2048, 512]; each core then computes
its 512 output columns for ALL rows (out[t, i*512:(i+1)*512]) with no
further collective. The host concatenates column blocks.

Dataflow per core (everything transposed-by-design, no PE transposes):
  proj:  qT/kT [d=128, T] = w-tile.T @ xT-tile, bf16 weights/x, fp32 PSUM,
         3 PSUM banks (two half-sweeps per tq-block; x streamed twice;
         v-half FIRST so its DRAM transpose round-trip hides)
  RoPE:  ACT does the half-swap, DVE the cos/sin muls; outputs bf16
  vT:    staged to DRAM bf16, read back via DMA-transpose -> v [t, d]
  sT  [tk, tq] = k-tile @ qT      (bf16)
  pT  = exp(sT/sqrt(d)) (ACT, bf16 out) * causal-mask (DVE)
  l   = ones.T @ pT               (M=1 matmul, fp32 PSUM accum over tk)
  oT  [d, tq] = v.T @ pT          (fp32 PSUM accum over tk)
  oT_norm = oT * (1/l)            (approx-recip + partition-broadcast), bf16
  oproj: out[t, 0:512] += agT-tile.T @ wo-tile  (bf16, fp32 PSUM, 32 k-tiles)
All matmuls bf16. Bulk DMAs are batched into >=0.5 MiB strided transfers
(weights/x/gathered/out) to amortize the ~0.6us per-DMA issue cost and hit
line-rate; issue is split across both HWDGE queues plus GpSimd SWDGE.
"""

import numpy as np

H, KV, HD, HID = 32, 8, 128, 4096
T = 2048
N_CORES = 8
QH = H // KV            # 4 query heads per core
DQ = QH * HD            # 512
KT = HID // 128         # 32 contraction tiles for projections
TN = T // 128           # 16 sequence tiles
G = 4                   # tq groups of 512
GW = T // G             # 512
NO = HID // N_CORES     # 512 output columns per core
XC = 8                  # k-tiles per x chunk DMA
WQC = 4                 # k-tiles per wq chunk DMA

THETA, FACTOR, HI_FF, LO_FF, ORIG_MAX = 500000.0, 8.0, 4.0, 1.0, 8192

_CACHE = {}


def _rope_tables():
    inv = 1.0 / (THETA ** (np.arange(0, HD, 2, dtype=np.float64) / HD))
    wavelen = 2.0 * np.pi / inv
    low_wl = ORIG_MAX / LO_FF
    high_wl = ORIG_MAX / HI_FF
    smooth = (ORIG_MAX / wavelen - LO_FF) / (HI_FF - LO_FF)
    scaled = np.where(wavelen > low_wl, inv / FACTOR, inv)
    mid = (wavelen <= low_wl) & (wavelen >= high_wl)
    scaled = np.where(mid, (1 - smooth) * inv / FACTOR + smooth * inv, scaled)
    inv32 = scaled.astype(np.float32)
    pos = np.arange(T, dtype=np.float32)
    freqs = pos[:, None] * inv32[None, :]          # [T, 64]
    emb = np.concatenate([freqs, freqs], axis=-1)  # [T, 128]
    cosT = np.ascontiguousarray(np.cos(emb).T)     # [128, T]
    sinT = np.ascontiguousarray(np.sin(emb).T)
    return cosT, sinT


def _causal_masks():
    # pT tile is [tk(part) 128, tq(free) 512]; within a tq-group the diagonal
    # tile sits at block v (=tk_tile - 4*g). keep where tq >= tk.
    tri = np.triu(np.ones((128, 128), dtype=np.float32))
    masks = np.zeros((4, 128, 512), dtype=np.float32)
    for v in range(4):
        for c in range(4):
            if c > v:
                masks[v, :, c * 128:(c + 1) * 128] = 1.0
            elif c == v:
                masks[v, :, c * 128:(c + 1) * 128] = tri
    return masks


def _build_program():
    import concourse.bacc as bacc
    import concourse.mybir as mybir
    from concourse.tile import TileContext

    f32 = mybir.dt.float32
    bf16 = mybir.dt.bfloat16
    EXPF = mybir.ActivationFunctionType.Exp

    nc = bacc.Bacc("TRN2", target_bir_lowering=False, debug=False,
                   num_devices=N_CORES)

    # weight DRAM tensors are pre-tiled on the host into [128, ...] layouts
    # so each loads with ONE contiguous line-rate DMA.
    xT = nc.dram_tensor("xT", [HID, T], bf16, kind="ExternalInput")
    wqd = nc.dram_tensor("wq", [128, KT * DQ], bf16, kind="ExternalInput")
    wkd = nc.dram_tensor("wk", [128, KT * HD], bf16, kind="ExternalInput")
    wvd = nc.dram_tensor("wv", [128, KT * HD], bf16, kind="ExternalInput")
    wod = nc.dram_tensor("wo", [128, KT * NO], bf16, kind="ExternalInput")
    cosd = nc.dram_tensor("cosT", [HD, T], f32, kind="ExternalInput")
    sind = nc.dram_tensor("sinT", [HD, T], f32, kind="ExternalInput")
    maskd = nc.dram_tensor("masks", [4, HD, GW], bf16, kind="ExternalInput")
    onesd = nc.dram_tensor("ones", [128, 1], bf16, kind="ExternalInput")
    outd = nc.dram_tensor("out", [T, NO], f32, kind="ExternalOutput")

    vstage = nc.dram_tensor("vstage", [HD, T], bf16)
    # one AllGather per (tq-block, head-pair): in [256,512] -> out [2048,512]
    ag_ins = [[nc.dram_tensor(f"ag_in{g}_{p}", [2 * HD, GW], bf16)
               for p in range(2)] for g in range(G)]
    ag_outs = [[nc.dram_tensor(f"ag_out{g}_{p}", [2 * HD * N_CORES, GW], bf16,
                               addr_space="Shared") for p in range(2)]
               for g in range(G)]
    wrm_in = nc.dram_tensor("wrm_in", [128, 16], bf16)
    wrm_out = nc.dram_tensor("wrm_out", [1024, 16], bf16, addr_space="Shared")

    scale = float(1.0 / np.sqrt(HD))

    with TileContext(nc) as tc:
        with (
            tc.tile_pool(name="const", bufs=1) as cpool,
            tc.tile_pool(name="wres", bufs=1) as wres,
            tc.tile_pool(name="stream", bufs=2) as stp,
            tc.tile_pool(name="qkv", bufs=2) as qkv,
            tc.tile_pool(name="kvres", bufs=1) as kvres,
            tc.tile_pool(name="rope", bufs=2) as rtp,
            tc.tile_pool(name="pt", bufs=4) as ptp,
            tc.tile_pool(name="norm", bufs=2) as nrm,
            tc.tile_pool(name="agbuf", bufs=1) as agp,
            tc.tile_pool(name="obuf", bufs=1) as obp,
            tc.tile_pool(name="ppsum", bufs=3, space="PSUM") as pps,
            tc.tile_pool(name="spsum", bufs=2, space="PSUM") as sps_pool,
            tc.tile_pool(name="opsum", bufs=1, space="PSUM") as ops_pool,
            tc.tile_pool(name="lpsum", bufs=1, space="PSUM") as lps_pool,
            tc.tile_pool(name="ojpsum", bufs=1, space="PSUM") as ojp,
        ):
            ones = cpool.tile([128, 1], bf16, tag="ones")
            mtiles = [cpool.tile([HD, GW], bf16, tag=f"mask{v}", name=f"mask{v}")
                      for v in range(4)]
            # resident weights; wq in 4-ktile chunks (fast first arrival),
            # wk/wv/wo as single big tiles loaded with one DMA each.
            wq_c = [wres.tile([128, WQC * DQ], bf16, tag=f"wq{c}", name=f"wq{c}")
                    for c in range(KT // WQC)]
            wkb = wres.tile([128, KT * HD], bf16, tag="wkb")
            wvb = wres.tile([128, KT * HD], bf16, tag="wvb")
            wob = wres.tile([128, KT * NO], bf16, tag="wob")
            kT_t = [kvres.tile([128, GW], bf16, tag=f"kT{g}", name=f"kT{g}") for g in range(G)]
            v_t = [kvres.tile([128, 128], bf16, tag=f"v{j}", name=f"v{j}") for j in range(TN)]

            warm = rtp.tile([1, 2], f32, tag="warm")
            nc.vector.memset(warm[:], 0.0)
            nc.scalar.activation(warm[:], warm[:], EXPF)
            wrs = rtp.tile([128, 16], bf16, tag="wrs")
            nc.vector.memset(wrs[:], 0.0)
            nc.gpsimd.dma_start(wrm_in[:], wrs[:])
            nc.gpsimd.collective_compute(
                "AllGather", mybir.AluOpType.bypass,
                replica_groups=[list(range(N_CORES))],
                ins=[wrm_in[:]], outs=[wrm_out[:]],
            )

            def wq_ap(k, h):
                c, kk = k // WQC, k % WQC
                return wq_c[c][:, kk * DQ + h * 128:kk * DQ + (h + 1) * 128]

            def rope_drain(dst, src, cg, sg):
                # dst[bf16] = src*cos + rotate_half(src)*sin ; src is fp32 PSUM
                rot = rtp.tile([128, GW], f32, tag="rot")
                nc.scalar.mul(rot[0:64, :], src[64:128, :], -1.0)
                nc.scalar.copy(rot[64:128, :], src[0:64, :])
                t1 = rtp.tile([128, GW], f32, tag="t1")
                nc.vector.tensor_mul(t1[:], src[:], cg[:])
                nc.vector.tensor_mul(rot[:], rot[:], sg[:])
                nc.vector.tensor_add(dst, t1[:], rot[:])

            ag_tiles = {}

            def oproj_fetch(g):
                ab = []
                for p in range(2):
                    at = agp.tile([128, 16 * GW], bf16, tag=f"ag{p}",
                                  name=f"ag{g}_{p}")
                    nc.sync.dma_start(
                        at[:].rearrange("p (b t) -> p b t", b=16),
                        ag_outs[g][p][:].rearrange("(b p) t -> p b t", p=128))
                    ab.append(at)
                ag_tiles[g] = ab

            def oproj_compute(g):
                # out[t, :NO] for t-block g from gathered attnT (all heads).
                # pair-p block b=2c+hh holds global head 4c+2p+hh -> wo
                # k-tile 4c+2p+hh.
                ab = ag_tiles.pop(g)
                order = [(p, c, hh) for p in range(2) for c in range(N_CORES)
                         for hh in range(2)]
                ob = obp.tile([128, 4 * NO], f32, tag="ob")
                for t in range(4):
                    opj = ojp.tile([128, NO], f32, tag="oj", name=f"oj{g}_{t}")
                    for fi, (p, c, hh) in enumerate(order):
                        lhs = ab[p][:, (2 * c + hh) * GW + t * 128:
                                    (2 * c + hh) * GW + (t + 1) * 128]
                        kw = 4 * c + 2 * p + hh
                        nc.tensor.matmul(opj[:], lhs, wob[:, kw * NO:(kw + 1) * NO],
                                         start=(fi == 0), stop=(fi == 31))
                    nc.vector.tensor_copy(ob[:, t * NO:(t + 1) * NO], opj[:])
                nc.scalar.dma_start(
                    outd[g * GW:(g + 1) * GW, :].rearrange("(tt p) n -> p tt n",
                                                           p=128),
                    ob[:].rearrange("p (tt n) -> p tt n", tt=4))

            qT_cur = [None] * QH   # per-g rotating qT tiles
            for g in range(G):
                gs = slice(g * GW, (g + 1) * GW)
                # ---- projections: two half-sweeps over 3 PSUM banks.
                # v-half FIRST: its DRAM round-trip hides under the k-half.
                for half in range(2):
                    h0, h1 = (2, 3) if half == 0 else (0, 1)
                    pp = [pps.tile([128, GW], f32, tag="pp", name=f"pp{g}_{half}_{_i}")
                          for _i in range(3)]
                    for k in range(KT):
                        if k % XC == 0:
                            xt = stp.tile([128, XC * GW], bf16, tag="xt")
                            nc.sync.dma_start(
                                xt[:].rearrange("p (c t) -> p c t", c=XC),
                                xT[k * 128:(k + XC) * 128, gs].rearrange(
                                    "(c p) t -> p c t", p=128))
                        if half == 0 and k == 16:
                            cg = rtp.tile([HD, GW], f32, tag="cosg")
                            sg = rtp.tile([HD, GW], f32, tag="sing")
                            nc.sync.dma_start(cg[:], cosd[:, gs])
                            nc.sync.dma_start(sg[:], sind[:, gs])
                        if half == 1 and k == 8:
                            for ts in range(4):
                                jj = 4 * g + ts
                                nc.sync.dma_start_transpose(
                                    v_t[jj][:], vstage[:, jj * 128:(jj + 1) * 128])
                        if g == 0 and half == 0 and k % WQC == 0:
                            nc.scalar.dma_start(
                                wq_c[k // WQC][:],
                                wqd[:, k * DQ:(k + WQC) * DQ])
                            if k == 0:
                                nc.scalar.dma_start(wvb[:], wvd[:])
                            if k == WQC:
                                nc.scalar.dma_start(wkb[:], wkd[:])
                        xs = xt[:, (k % XC) * GW:(k % XC + 1) * GW]
                        st, sp = (k == 0), (k == KT - 1)
                        nc.tensor.matmul(pp[1][:], wq_ap(k, h0), xs, start=st, stop=sp)
                        nc.tensor.matmul(pp[2][:], wq_ap(k, h1), xs, start=st, stop=sp)
                        wkv = (wvb if half == 0 else wkb)[:, k * HD:(k + 1) * HD]
                        nc.tensor.matmul(pp[0][:], wkv, xs, start=st, stop=sp)
                    # drains (v half first: DRAM round-trip ahead of it)
                    if half == 0:
                        vt = qkv.tile([128, GW], bf16, tag="vT")
                        nc.scalar.copy(vt[:], pp[0][:])
                        nc.gpsimd.dma_start(vstage[:, gs], vt[:])
                    for i, h in enumerate((h0, h1)):
                        qt = qkv.tile([128, GW], bf16, tag=f"qT{h}", name=f"qT{h}_{g}")
                        rope_drain(qt[:], pp[1 + i], cg, sg)
                        qT_cur[h] = qt
                    if half == 1:
                        rope_drain(kT_t[g][:], pp[0], cg, sg)
                    if g == 0 and half == 1:
                        nc.sync.dma_start(ones[:], onesd[:])
                        for v in range(4):
                            nc.sync.dma_start(mtiles[v][:], maskd[v])

                if g >= 2:
                    oproj_compute(g - 2)

                # ---- attention for tq-block g; AG fires per head-pair ----
                nj = 4 * g + 4
                for h in range(QH):
                    ops_ = ops_pool.tile([128, GW], f32, tag="op")
                    lps = lps_pool.tile([1, GW], f32, tag="lp")

                    def lo_flush(jj, c0, pt):
                        nc.tensor.matmul(lps[:, c0:], ones[:], pt[:, c0:],
                                         start=(jj == 0), stop=(jj == nj - 1))
                        nc.tensor.matmul(ops_[:, c0:], v_t[jj][:], pt[:, c0:],
                                         start=(jj == 0), stop=(jj == nj - 1))

                    prev = None
                    for j in range(nj):
                        c0 = (j - 4 * g) * 128 if j >= 4 * g else 0
                        sps = sps_pool.tile([128, GW], f32, tag="sp")
                        nc.tensor.matmul(sps[:, c0:], kT_t[j // 4][:, (j % 4) * 128:(j % 4 + 1) * 128],
                                         qT_cur[h][:, c0:], start=True, stop=True)
                        pt = ptp.tile([128, GW], bf16, tag="pt")
                        nc.scalar.activation(pt[:, c0:], sps[:, c0:], EXPF, scale=scale)
                        if j >= 4 * g:
                            nc.vector.tensor_mul(pt[:, c0:], pt[:, c0:],
                                                 mtiles[j - 4 * g][:, c0:])
                        if prev is not None:
                            lo_flush(*prev)
                        prev = (j, c0, pt)
                    lo_flush(*prev)
                    ls = nrm.tile([1, GW], f32, tag="ls")
                    nc.vector.reciprocal_approx_fast(ls[:], lps[:])
                    lb = nrm.tile([128, GW], f32, tag="lb")
                    nc.gpsimd.partition_broadcast(lb[:], ls[:])
                    ot = qkv.tile([128, GW], bf16, tag=f"oT{h}")
                    nc.vector.tensor_mul(ot[:], ops_[:], lb[:])
                    nc.gpsimd.dma_start(ag_ins[g][h // 2][(h % 2) * 128:(h % 2 + 1) * 128, :],
                                        ot[:])
                    if h % 2 == 1:
                        p = h // 2
                        nc.gpsimd.collective_compute(
                            "AllGather", mybir.AluOpType.bypass,
                            replica_groups=[list(range(N_CORES))],
                            ins=[ag_ins[g][p][:]], outs=[ag_outs[g][p][:]],
                        )

                if g == 0:
                    nc.gpsimd.dma_start(wob[:], wod[:])
                if g >= 1:
                    oproj_fetch(g - 1)
            oproj_compute(G - 2)
            oproj_fetch(G - 1)
            oproj_compute(G - 1)

    nc.compile()
    return nc


def _get_program():
    if "nc" not in _CACHE:
        _CACHE["nc"] = _build_program()
    return _CACHE["nc"]


def _tile_rows(w, inner):
    # [HID, inner] -> [128, KT*inner]: k-tile k's rows land at cols k*inner
    return np.ascontiguousarray(
        w.reshape(KT, 128, inner).transpose(1, 0, 2).reshape(128, KT * inner))


def kernel(x, wq, wk, wv, wo):
    from concourse.bass_utils import run_bass_kernel_spmd
    from ml_dtypes import bfloat16

    nc = _get_program()

    x2 = np.asarray(x, dtype=np.float32).reshape(T, HID)
    xT = np.ascontiguousarray(x2.T).astype(bfloat16)
    cosT, sinT = _rope_tables()
    masks = _causal_masks().astype(bfloat16)
    ones = np.ones((128, 1), dtype=np.float32).astype(bfloat16)

    wq = np.asarray(wq, dtype=np.float32)
    wk = np.asarray(wk, dtype=np.float32)
    wv = np.asarray(wv, dtype=np.float32)
    wo = np.asarray(wo, dtype=np.float32)

    in_maps = []
    for i in range(N_CORES):
        in_maps.append({
            "xT": xT,
            "wq": _tile_rows(wq[:, i * DQ:(i + 1) * DQ], DQ).astype(bfloat16),
            "wk": _tile_rows(wk[:, i * HD:(i + 1) * HD], HD).astype(bfloat16),
            "wv": _tile_rows(wv[:, i * HD:(i + 1) * HD], HD).astype(bfloat16),
            "wo": _tile_rows(wo[:, i * NO:(i + 1) * NO], NO).astype(bfloat16),
            "cosT": cosT,
            "sinT": sinT,
            "masks": masks,
            "ones": ones,
        })

    _CACHE["last_in_maps"] = in_maps
    res = run_bass_kernel_spmd(nc, in_maps, list(range(N_CORES)))
    _CACHE["last_result"] = res
    out = np.empty((T, HID), dtype=np.float32)
    for i in range(N_CORES):
        out[:, i * NO:(i + 1) * NO] = res.results[i]["out"]
    return out.reshape(1, T, HID)
